# revision 2
# baseline (speedup 1.0000x reference)
"""BorderLoss TRN2 kernel v2.

Per element: loss = softplus((1-2y)*x); w = 1 + border; out = mean(loss*w).
border via box-count: with yp = 1-2y in {+-1}, t = 3x3 box sum of yp
(OOB=0), cnt = in-bounds cells; border <=> |t| <= cnt-2. Edge rows/cols
scaled (rows x1.5 via tri lhsT; cols asymmetric 1.5*yp_e + yp_n via one
STT) so non-border |t| is 9 (interior/edge-row), 7.5 (edge-col/corner)
while border |t| <= 7; threshold 7.25 separates universally.

total_img = 2*sum(l) - sum(l*G), G = [|t| >= THR].
sum(l) from Ln accum_out; sum(l*G) via STTs:
  cols [0:CA): ACT Abs(t)->a, STT (a is_ge THR)*l accum
  cols [CA:2048): STT (t is_ge THR)*l + STT (t is_le -THR)*l accums

Layout '(b p)': image row r = b*128 + p; tile col c = b*512 + w.
Host preps x,yp as [8, 128, 2048] bf16 per core (cast+reshape only).

Engines: ACT: Exp x8 grouped (1 table load), then Ln/Abs (1 load);
Pool: h2 = yp_l + yp_r (shifted TT); DVE: z, h3 = h2+yp, edge STTs, u2
STTs; PE: 10 matmuls/img (tri variants + U/L single-entry).
"""

import sys
import numpy as np

if "/opt/trn_rl_repo" not in sys.path:
    sys.path.insert(0, "/opt/trn_rl_repo")

H = W = 512
P = 128
NB = 4
FI = 2048
N_CORES = 8
NI = 8               # images per core
NACC = 2             # sum(l), u2
THR = 7.25

_CACHE = {}


def _consts():
    import ml_dtypes
    bf = ml_dtypes.bfloat16
    tri0 = np.zeros((P, P), dtype=np.float64)
    for k in range(P):
        tri0[k, max(0, k - 1):min(P, k + 2)] = 1.0
    t_first = tri0.copy()
    t_first[:, 0] *= 1.5          # out-row 0 of block 0 = image row 0
    t_last = tri0.copy()
    t_last[:, P - 1] *= 1.5       # out-row 127 of block 3 = image row 511
    u0 = np.zeros((P, P), dtype=np.float64)
    u0[0, P - 1] = 1.0            # next block row 0 -> out row 127
    l0 = np.zeros((P, P), dtype=np.float64)
    l0[P - 1, 0] = 1.0            # prev block row 127 -> out row 0
    tri = np.zeros((P, 5 * P), dtype=bf)
    for i, m in enumerate((t_first, tri0, t_last, u0, l0)):
        tri[:, i * P:(i + 1) * P] = m.astype(bf)
    return tri


def _build():
    import concourse.bass as bass
    import concourse.bacc as bacc
    import concourse.tile as tile
    from concourse import mybir

    f32 = mybir.dt.float32
    bf16 = mybir.dt.bfloat16
    Alu = mybir.AluOpType
    Act = mybir.ActivationFunctionType

    nc = bacc.Bacc(None, target_bir_lowering=False)
    x_d = nc.dram_tensor("x", [NI, P, FI], bf16, kind="ExternalInput")
    y_d = nc.dram_tensor("y", [NI, P, FI], bf16, kind="ExternalInput")
    tri_d = nc.dram_tensor("tri", [P, 5 * P], bf16, kind="ExternalInput")
    acc_d = nc.dram_tensor("acc", [P, NI * NACC], f32, kind="ExternalOutput")

    with tile.TileContext(nc) as tc:
        with (
            tc.tile_pool(name="consts", bufs=1) as cpool,
            tc.tile_pool(name="io", bufs=4) as io,
            tc.tile_pool(name="work", bufs=4) as work,
            tc.tile_pool(name="accp", bufs=1) as apool,
            tc.tile_pool(name="ps", bufs=2, space=bass.MemorySpace.PSUM) as pp,
        ):
            tri = cpool.tile([P, 5 * P], bf16)
            accs = apool.tile([P, NI * NACC], f32)
            nc.vector.memset(accs[:], 0.0)
            # pin the one table holding Exp+Ln+Abs: no reloads ever
            nc.scalar.add_instruction(mybir.InstLoadActFuncSet(
                name=nc.get_next_instruction_name(), act_func_set_id=6,
                ins=[], outs=[]))

            pend = {}

            def readout(j):
                ps_j, lt_j = pend.pop(j)
                b0 = j * NACC
                at = work.tile([P, FI], bf16, tag="at")
                nc.scalar.activation(at[:], ps_j[:], Act.Abs)
                nc.vector.scalar_tensor_tensor(
                    at[:], at[:], THR, lt_j[:], Alu.is_ge, Alu.mult,
                    accum_out=accs[:, b0 + 1:b0 + 2])

            for i in range(NI):
                a0 = i * NACC
                xb = io.tile([P, FI], bf16, tag="xb")
                yp = io.tile([P, FI], bf16, tag="yp")
                nc.sync.dma_start(xb[:], x_d[i])
                nc.sync.dma_start(yp[:], y_d[i])
                if i == 0:
                    nc.sync.dma_start(tri[:], tri_d[:])

                zb = io.tile([P, FI], bf16, tag="zb")
                nc.vector.tensor_tensor(zb[:], yp[:], xb[:], Alu.mult)
                ez = work.tile([P, FI], bf16, tag="ez")
                nc.scalar.activation(ez[:], zb[:], Act.Exp)

                h = work.tile([P, FI], bf16, tag="h")
                nc.vector.tensor_tensor(h[:, 1:FI - 1], yp[:, 0:FI - 2],
                                        yp[:, 2:FI], Alu.add)
                nc.vector.tensor_tensor(h[:, 1:FI - 1], h[:, 1:FI - 1],
                                        yp[:, 1:FI - 1], Alu.add)
                # edge cols w in {0,511}: h = 1.5*yp_edge + yp_neighbor
                h3 = h.rearrange("p (b w) -> p b w", w=W)
                y3 = yp.rearrange("p (b w) -> p b w", w=W)
                nc.vector.scalar_tensor_tensor(
                    h3[:, :, 0:W:W - 1], y3[:, :, 0:W:W - 1], 1.5,
                    y3[:, :, 1:W - 1:W - 3], Alu.mult, Alu.add)

                ps = pp.tile([P, FI], f32, tag="ps")
                for b in range(NB):
                    o = ps[:, b * W:(b + 1) * W]
                    t_idx = 0 if b == 0 else (2 if b == NB - 1 else 1)
                    mms = [(tri[:, t_idx * P:(t_idx + 1) * P],
                            h[:, b * W:(b + 1) * W])]
                    if b > 0:
                        mms.append((tri[:, 4 * P:5 * P],
                                    h[:, (b - 1) * W:b * W]))
                    if b < NB - 1:
                        mms.append((tri[:, 3 * P:4 * P],
                                    h[:, (b + 1) * W:(b + 2) * W]))
                    for k, (ltm, r) in enumerate(mms):
                        nc.tensor.matmul(o, ltm, r, start=(k == 0),
                                         stop=(k == len(mms) - 1))

                lt = work.tile([P, FI], bf16, tag="lt")
                nc.scalar.activation(lt[:], ez[:], Act.Ln, bias=1.0,
                                     accum_out=accs[:, a0:a0 + 1])

                # deferred readout of image i-1 (keeps ACT FIFO off the
                # PE critical path; psum released one image later)
                if i > 0:
                    readout(i - 1)
                pend[i] = (ps, lt)

            readout(NI - 1)
            nc.sync.dma_start(acc_d[:], accs[:])

    nc.compile()
    return nc


def _get_nc():
    if "nc" not in _CACHE:
        _CACHE["nc"] = _build()
    return _CACHE["nc"]


def _prep(x, y):
    import ml_dtypes
    bf = ml_dtypes.bfloat16
    n = x.shape[0]
    xb = np.ascontiguousarray(
        x.reshape(n, NB, P, W).transpose(0, 2, 1, 3).reshape(n, P, FI)
    ).astype(bf)
    yp = np.ascontiguousarray(
        (1 - 2 * y).reshape(n, NB, P, W).transpose(0, 2, 1, 3)
        .reshape(n, P, FI)).astype(bf)
    return xb, yp


def kernel(x, y):
    from concourse import bass_utils

    n = x.shape[0]
    assert n == N_CORES * NI
    nc = _get_nc()
    tri = _consts()
    xb, yp = _prep(np.asarray(x, dtype=np.float32),
                   np.asarray(y, dtype=np.int64).astype(np.float32))
    in_maps = [
        {"x": xb[c * NI:(c + 1) * NI], "y": yp[c * NI:(c + 1) * NI],
         "tri": tri}
        for c in range(N_CORES)
    ]
    res = bass_utils.run_bass_kernel_spmd(nc, in_maps,
                                          core_ids=list(range(N_CORES)))
    total = 0.0
    for r in res.results:
        a = r["acc"].reshape(P, NI, NACC).astype(np.float64)
        total += 2.0 * a[:, :, 0].sum() - a[:, :, 1].sum()
    return np.float32(total / (n * H * W))


# revision 4
# speedup vs baseline: 1.1602x; 1.1602x over previous
"""BorderLoss TRN2 kernel v2.

Per element: loss = softplus((1-2y)*x); w = 1 + border; out = mean(loss*w).
border via box-count: with yp = 1-2y in {+-1}, t = 3x3 box sum of yp
(OOB=0), cnt = in-bounds cells; border <=> |t| <= cnt-2. Edge rows/cols
scaled (rows x1.5 via tri lhsT; cols asymmetric 1.5*yp_e + yp_n via one
STT) so non-border |t| is 9 (interior/edge-row), 7.5 (edge-col/corner)
while border |t| <= 7; threshold 7.25 separates universally.

total_img = 2*sum(l) - sum(l*G), G = [|t| >= THR].
sum(l) from Ln accum_out; sum(l*G) via STTs:
  cols [0:CA): ACT Abs(t)->a, STT (a is_ge THR)*l accum
  cols [CA:2048): STT (t is_ge THR)*l + STT (t is_le -THR)*l accums

Layout '(b p)': image row r = b*128 + p; tile col c = b*512 + w.
Host preps x,yp as [8, 128, 2048] bf16 per core (cast+reshape only).

Engines: ACT: Exp x8 grouped (1 table load), then Ln/Abs (1 load);
Pool: h2 = yp_l + yp_r (shifted TT); DVE: z, h3 = h2+yp, edge STTs, u2
STTs; PE: 10 matmuls/img (tri variants + U/L single-entry).
"""

import sys
import numpy as np

if "/opt/trn_rl_repo" not in sys.path:
    sys.path.insert(0, "/opt/trn_rl_repo")

H = W = 512
P = 128
NB = 4
FI = 2048
N_CORES = 8
NI = 8               # images per core
NACC = 2             # sum(l), u2
THR = 7.25

_CACHE = {}


def _consts():
    import ml_dtypes
    bf = ml_dtypes.bfloat16
    tri0 = np.zeros((P, P), dtype=np.float64)
    for k in range(P):
        tri0[k, max(0, k - 1):min(P, k + 2)] = 1.0
    t_first = tri0.copy()
    t_first[:, 0] *= 1.5          # out-row 0 of block 0 = image row 0
    t_last = tri0.copy()
    t_last[:, P - 1] *= 1.5       # out-row 127 of block 3 = image row 511
    u0 = np.zeros((P, P), dtype=np.float64)
    u0[0, P - 1] = 1.0            # next block row 0 -> out row 127
    l0 = np.zeros((P, P), dtype=np.float64)
    l0[P - 1, 0] = 1.0            # prev block row 127 -> out row 0
    tri = np.zeros((P, 5 * P), dtype=bf)
    for i, m in enumerate((t_first, tri0, t_last, u0, l0)):
        tri[:, i * P:(i + 1) * P] = m.astype(bf)
    return tri


def _build():
    import concourse.bass as bass
    import concourse.bacc as bacc
    import concourse.tile as tile
    from concourse import mybir

    f32 = mybir.dt.float32
    bf16 = mybir.dt.bfloat16
    Alu = mybir.AluOpType
    Act = mybir.ActivationFunctionType

    nc = bacc.Bacc(None, target_bir_lowering=False)
    x_d = nc.dram_tensor("x", [NI, P, FI], bf16, kind="ExternalInput")
    y_d = nc.dram_tensor("y", [NI, P, FI], bf16, kind="ExternalInput")
    tri_d = nc.dram_tensor("tri", [P, 5 * P], bf16, kind="ExternalInput")
    acc_d = nc.dram_tensor("acc", [P, NI * NACC], f32, kind="ExternalOutput")

    with tile.TileContext(nc) as tc:
        with (
            tc.tile_pool(name="consts", bufs=1) as cpool,
            tc.tile_pool(name="io", bufs=4) as io,
            tc.tile_pool(name="work", bufs=4) as work,
            tc.tile_pool(name="accp", bufs=1) as apool,
            tc.tile_pool(name="ps", bufs=2, space=bass.MemorySpace.PSUM) as pp,
        ):
            tri = cpool.tile([P, 5 * P], bf16)
            accs = apool.tile([P, NI * NACC], f32)
            nc.vector.memset(accs[:], 0.0)
            warm = apool.tile([P, 2], bf16)
            nc.gpsimd.tensor_scalar(warm[:, 0:1], accs[:, 0:1], 1.0, 0.0,
                                    Alu.mult, Alu.add)
            # pin the one table holding Exp+Ln+Abs: no reloads ever
            nc.scalar.add_instruction(mybir.InstLoadActFuncSet(
                name=nc.get_next_instruction_name(), act_func_set_id=6,
                ins=[], outs=[]))

            pend = {}

            def readout(j):
                ps_j, lt_j = pend.pop(j)
                b0 = j * NACC
                at = work.tile([P, FI], bf16, tag="at")
                nc.scalar.activation(at[:], ps_j[:], Act.Abs)
                nc.vector.scalar_tensor_tensor(
                    at[:], at[:], THR, lt_j[:], Alu.is_ge, Alu.mult,
                    accum_out=accs[:, b0 + 1:b0 + 2])

            for i in range(NI):
                a0 = i * NACC
                xb = io.tile([P, FI], bf16, tag="xb")
                yp = io.tile([P, FI], bf16, tag="yp")
                nc.sync.dma_start(xb[:], x_d[i])
                nc.sync.dma_start(yp[:], y_d[i])
                if i == 0:
                    nc.sync.dma_start(tri[:], tri_d[:])

                zb = io.tile([P, FI], bf16, tag="zb")
                nc.vector.tensor_tensor(zb[:], yp[:], xb[:], Alu.mult)
                ez = work.tile([P, FI], bf16, tag="ez")
                nc.scalar.activation(ez[:], zb[:], Act.Exp)

                h = work.tile([P, FI], bf16, tag="h")
                nc.vector.tensor_tensor(h[:, 1:FI - 1], yp[:, 0:FI - 2],
                                        yp[:, 2:FI], Alu.add)
                nc.vector.tensor_tensor(h[:, 1:FI - 1], h[:, 1:FI - 1],
                                        yp[:, 1:FI - 1], Alu.add)
                # edge cols w in {0,511}: h = 1.5*yp_edge + yp_neighbor
                h3 = h.rearrange("p (b w) -> p b w", w=W)
                y3 = yp.rearrange("p (b w) -> p b w", w=W)
                he = h3[:, :, 0:W:W - 1]
                nc.gpsimd.tensor_tensor(he, y3[:, :, 0:W:W - 1],
                                        y3[:, :, 1:W - 1:W - 3], Alu.add)
                nc.gpsimd.tensor_scalar(he, he, 1.5, 0.0, Alu.mult, Alu.add)

                ps = pp.tile([P, FI], f32, tag="ps")
                for b in range(NB):
                    o = ps[:, b * W:(b + 1) * W]
                    t_idx = 0 if b == 0 else (2 if b == NB - 1 else 1)
                    mms = [(tri[:, t_idx * P:(t_idx + 1) * P],
                            h[:, b * W:(b + 1) * W])]
                    if b > 0:
                        mms.append((tri[:, 4 * P:5 * P],
                                    h[:, (b - 1) * W:b * W]))
                    if b < NB - 1:
                        mms.append((tri[:, 3 * P:4 * P],
                                    h[:, (b + 1) * W:(b + 2) * W]))
                    for k, (ltm, r) in enumerate(mms):
                        nc.tensor.matmul(o, ltm, r, start=(k == 0),
                                         stop=(k == len(mms) - 1))

                lt = work.tile([P, FI], bf16, tag="lt")
                nc.scalar.activation(lt[:], ez[:], Act.Ln, bias=1.0,
                                     accum_out=accs[:, a0:a0 + 1])

                # deferred readout of image i-1 (keeps ACT FIFO off the
                # PE critical path; psum released one image later)
                if i > 0:
                    readout(i - 1)
                pend[i] = (ps, lt)

            readout(NI - 1)
            nc.sync.dma_start(acc_d[:], accs[:])

    nc.compile()
    return nc


def _get_nc():
    if "nc" not in _CACHE:
        _CACHE["nc"] = _build()
    return _CACHE["nc"]


def _prep(x, y):
    import ml_dtypes
    bf = ml_dtypes.bfloat16
    n = x.shape[0]
    xb = np.ascontiguousarray(
        x.reshape(n, NB, P, W).transpose(0, 2, 1, 3).reshape(n, P, FI)
    ).astype(bf)
    yp = np.ascontiguousarray(
        (1 - 2 * y).reshape(n, NB, P, W).transpose(0, 2, 1, 3)
        .reshape(n, P, FI)).astype(bf)
    return xb, yp


def kernel(x, y):
    from concourse import bass_utils

    n = x.shape[0]
    assert n == N_CORES * NI
    nc = _get_nc()
    tri = _consts()
    xb, yp = _prep(np.asarray(x, dtype=np.float32),
                   np.asarray(y, dtype=np.int64).astype(np.float32))
    in_maps = [
        {"x": xb[c * NI:(c + 1) * NI], "y": yp[c * NI:(c + 1) * NI],
         "tri": tri}
        for c in range(N_CORES)
    ]
    res = bass_utils.run_bass_kernel_spmd(nc, in_maps,
                                          core_ids=list(range(N_CORES)))
    total = 0.0
    for r in res.results:
        a = r["acc"].reshape(P, NI, NACC).astype(np.float64)
        total += 2.0 * a[:, :, 0].sum() - a[:, :, 1].sum()
    return np.float32(total / (n * H * W))
